# revision 1
# baseline (speedup 1.0000x reference)
"""Batched KNN (K=32) on 8 Trainium2 NeuronCores.

Exploits that `batch` is sorted: the 8 batch groups are contiguous blocks of
~1024 points, and KNN only needs within-block distances.  Core b gets block b:
it computes v = -d2 = 2*x@x.T - sq_i - sq_j for its [nb, nb] block (PE matmul
+ ACT bias + POOL broadcast add), masks the diagonal via gpsimd affine_select,
and extracts the top-32 (largest v = smallest d2) per row with 4 rounds of
DVE max8 / max_index / match_replace.
"""

import os
import sys

import numpy as np

for _p in ("/opt/trn_rl_repo", "/root/.axon_site/_ro/trn_rl_repo"):
    if os.path.isdir(_p) and _p not in sys.path:
        sys.path.append(_p)

K = 32
BIG = 1e30
N_CORES = 8

LAST_EXEC_NS = None

_NC_CACHE = {}


def _build_nc(W, T, D):
    import concourse.bass as bass
    from concourse import bacc, mybir
    from concourse.tile import TileContext

    f32 = mybir.dt.float32
    u32 = mybir.dt.uint32
    KC = D // 128
    assert D % 128 == 0

    P = T * 128
    nc = bacc.Bacc(None, target_bir_lowering=False)
    xt_d = nc.dram_tensor("xt", [D, W], f32, kind="ExternalInput")
    nsqc_d = nc.dram_tensor("nsqc", [128, T], f32, kind="ExternalInput")
    sqjn_d = nc.dram_tensor("sqjn", [1, W], f32, kind="ExternalInput")
    od_d = nc.dram_tensor("od", [P, K], f32, kind="ExternalOutput")
    oi_d = nc.dram_tensor("oi", [P, K], u32, kind="ExternalOutput")

    with TileContext(nc) as tc:
        with tc.tile_pool(name="const", bufs=1) as cpool, \
             tc.tile_pool(name="work", bufs=3) as wpool, \
             tc.tile_pool(name="outp", bufs=3) as opool, \
             tc.tile_pool(name="psum", bufs=2, space="PSUM") as ppool:
            xt_sb = []
            for k in range(KC):
                xk = cpool.tile([128, W], f32, tag=f"xt{k}")
                for c0 in range(0, W, 512):
                    cn = min(512, W - c0)
                    nc.sync.dma_start(
                        xk[:, c0:c0 + cn],
                        xt_d[k * 128:(k + 1) * 128, c0:c0 + cn])
                xt_sb.append(xk)
            nsqc_sb = cpool.tile([128, T], f32, tag="nsqc")
            nc.sync.dma_start(nsqc_sb[:, :], nsqc_d[:, :])
            sqjn_sb = cpool.tile([128, W], f32, tag="sqjn")
            nc.sync.dma_start(
                sqjn_sb[:, :], sqjn_d[0:1, :].to_broadcast((128, W)))

            for t in range(T):
                q0 = t * 128
                m = min(128, W - q0)
                ps = ppool.tile([128, W], f32, tag="ps")
                for k in range(KC):
                    for c0 in range(0, W, 512):
                        cn = min(512, W - c0)
                        nc.tensor.matmul(
                            ps[:m, c0:c0 + cn],
                            xt_sb[k][:, q0:q0 + m],
                            xt_sb[k][:, c0:c0 + cn],
                            start=(k == 0),
                            stop=(k == KC - 1),
                        )
                v = wpool.tile([128, W], f32, tag="v")
                # v = 2*dot - sq_i  (bias is per-partition -sq_i)
                nc.scalar.activation(
                    v[:m, :], ps[:m, :],
                    mybir.ActivationFunctionType.Identity,
                    bias=nsqc_sb[:m, t:t + 1], scale=2.0,
                )
                # v += -sq_j  (pad cols carry -BIG here)
                nc.gpsimd.tensor_add(v[:m, :], v[:m, :], sqjn_sb[:m, :])
                # diagonal (self) -> -BIG: keep where (q0 + p - c) != 0
                nc.gpsimd.affine_select(
                    out=v[:m, :], in_=v[:m, :],
                    compare_op=mybir.AluOpType.not_equal,
                    fill=-BIG, base=q0,
                    pattern=[[-1, W]], channel_multiplier=1,
                )
                vals = opool.tile([128, K], f32, tag="vals")
                inds = opool.tile([128, K], u32, tag="inds")
                for r in range(K // 8):
                    sl = slice(8 * r, 8 * r + 8)
                    nc.vector.max(out=vals[:m, sl], in_=v[:m, :])
                    nc.vector.max_index(
                        out=inds[:m, sl], in_max=vals[:m, sl], in_values=v[:m, :])
                    if r < K // 8 - 1:
                        nc.vector.match_replace(
                            out=v[:m, :], in_to_replace=vals[:m, sl],
                            in_values=v[:m, :], imm_value=-BIG)
                d2o = opool.tile([128, K], f32, tag="d2o")
                nc.scalar.activation(
                    d2o[:m, :], vals[:m, :],
                    mybir.ActivationFunctionType.Identity, scale=-1.0)
                nc.sync.dma_start(od_d[q0:q0 + m, :], d2o[:m, :])
                nc.sync.dma_start(oi_d[q0:q0 + m, :], inds[:m, :])
    nc.finalize()
    return nc


def kernel(x, batch):
    global LAST_EXEC_NS
    from concourse.bass_utils import run_bass_kernel_spmd

    x = np.ascontiguousarray(np.asarray(x), dtype=np.float32)
    b = np.asarray(batch)
    N, D = x.shape
    bounds = np.searchsorted(b, np.arange(N_CORES + 1))
    sizes = np.diff(bounds)
    W = max(128, int(-(-sizes.max() // 8)) * 8)
    T = max(1, int(-(-sizes.max() // 128)))

    key = (W, T, D)
    if key not in _NC_CACHE:
        _NC_CACHE[key] = _build_nc(W, T, D)
    nc = _NC_CACHE[key]

    in_maps = []
    for c in range(N_CORES):
        s, e = int(bounds[c]), int(bounds[c + 1])
        n = e - s
        xc = x[s:e]
        xt = np.zeros((D, W), np.float32)
        xt[:, :n] = xc.T
        sq = np.einsum("ij,ij->i", xc, xc, dtype=np.float32)
        sq_pad = np.zeros(T * 128, np.float32)
        sq_pad[:n] = sq
        nsqc = np.ascontiguousarray((-sq_pad).reshape(T, 128).T)
        row = np.full(W, -BIG, np.float32)
        row[:n] = -sq
        sqjn = row.reshape(1, W)
        in_maps.append({"xt": xt, "nsqc": nsqc, "sqjn": sqjn})

    trace = os.environ.get("KNN_TRACE", "0") == "1"
    res = run_bass_kernel_spmd(
        nc, in_maps, core_ids=list(range(N_CORES)), trace=trace)
    LAST_EXEC_NS = res.exec_time_ns

    out_d = np.empty((N, K), np.float32)
    out_i = np.empty((N, K), np.int32)
    for c in range(N_CORES):
        s, e = int(bounds[c]), int(bounds[c + 1])
        n = e - s
        if n == 0:
            continue
        out_d[s:e] = res.results[c]["od"][:n]
        out_i[s:e] = res.results[c]["oi"][:n].astype(np.int64) + s
    return out_d, out_i



# revision 2
# speedup vs baseline: 2.4111x; 2.4111x over previous
"""Batched KNN (K=32) on 8 Trainium2 NeuronCores — keyed top-k v5.

Keyed scheme (see kernel4): v = 2*dot - sq_i - sq_j is quantized to 0.5
steps by fp32 rounding in the 2^34 binade and packed with the column index
into an exact integer key q*2048 + j; DVE does 4x max8 + 3x match_replace
per 128-row tile and the host decodes distances + indices from the keys.

v5 on top of v4:
 - -sq_j/2 is folded into the gpsimd key-add operand (iota2[j] =
   j - round(2*sq_j)*2048, exact in fp32) — no k=2 bias matmuls.
 - xt is DMAed per 512-column chunk into separate tiles so the first
   matmul starts after ~256KB instead of the full transfer.
 - dummy matmuls + a dummy activation during the DMA wait warm the PE
   (HAM throttle) and pull the one-time ACT_TABLE_LOAD off the
   critical path.
"""

import os
import sys

import numpy as np

for _p in ("/opt/trn_rl_repo", "/root/.axon_site/_ro/trn_rl_repo"):
    if os.path.isdir(_p) and _p not in sys.path:
        sys.path.append(_p)

K = 32
BIG = 1e30
N_CORES = 8
C34 = float(2.0 ** 34)
C23 = float(2.0 ** 23)

LAST_EXEC_NS = None

_NC_CACHE = {}


def _build_nc(W, T, D):
    from concourse import bacc, mybir
    from concourse.tile import TileContext

    f32 = mybir.dt.float32
    bf16 = mybir.dt.bfloat16
    KC = D // 128
    assert D % 128 == 0

    P = T * 128
    nc = bacc.Bacc(None, target_bir_lowering=False)
    xt_d = nc.dram_tensor("xt", [D, W], bf16, kind="ExternalInput")
    biasp_d = nc.dram_tensor("biasp", [128, T], f32, kind="ExternalInput")
    dei_d = nc.dram_tensor("dei", [128, 256], bf16, kind="ExternalInput")
    iota2_d = nc.dram_tensor("iota2", [1, W], f32, kind="ExternalInput")
    ok_d = nc.dram_tensor("ok", [P, K], f32, kind="ExternalOutput")

    CC = [(c0, min(512, W - c0)) for c0 in range(0, W, 512)]

    with TileContext(nc) as tc:
        with tc.tile_pool(name="const", bufs=1) as cpool, \
             tc.tile_pool(name="w1p", bufs=4) as wpool, \
             tc.tile_pool(name="keyp", bufs=3) as kpool, \
             tc.tile_pool(name="outp", bufs=4) as opool, \
             tc.tile_pool(name="psum", bufs=2, space="PSUM") as ppool, \
             tc.tile_pool(name="scr", bufs=1, space="PSUM") as spool:
            # --- pull the one-time ACT_TABLE_LOAD off the critical path ---
            warm_sb = cpool.tile([2, 8], f32, tag="warm")
            nc.vector.memset(warm_sb[:, :], 0.0)
            warmo_sb = cpool.tile([2, 8], f32, tag="warmo")
            nc.scalar.activation(
                warmo_sb[:, :], warm_sb[:, :],
                mybir.ActivationFunctionType.Identity)

            # --- input DMAs, first-needed first ---
            xt_sb = [[None] * len(CC) for _ in range(KC)]
            for ci, (c0, cn) in enumerate(CC):
                for k in range(KC):
                    xkc = cpool.tile([128, cn], bf16, tag=f"xt{k}_{ci}")
                    xt_sb[k][ci] = xkc
            for k in range(KC):
                c0, cn = CC[0]
                nc.sync.dma_start(
                    xt_sb[k][0][:, :], xt_d[k * 128:(k + 1) * 128, c0:c0 + cn])
            dei_sb = cpool.tile([128, 256], bf16, tag="dei")
            nc.sync.dma_start(dei_sb[:, :], dei_d[:, :])
            biasp_sb = cpool.tile([128, T], f32, tag="biasp")
            nc.sync.dma_start(biasp_sb[:, :], biasp_d[:, :])
            for ci, (c0, cn) in enumerate(CC[1:], start=1):
                for k in range(KC):
                    nc.sync.dma_start(
                        xt_sb[k][ci][:, :],
                        xt_d[k * 128:(k + 1) * 128, c0:c0 + cn])
            iota_sb = cpool.tile([128, W], f32, tag="iota2")
            nc.sync.dma_start(
                iota_sb[:, :], iota2_d[0:1, :].to_broadcast((128, W)))
            bn34_sb = cpool.tile([128, 1], f32, tag="bn34")
            nc.vector.memset(bn34_sb[:, :], -C34)

            for t in range(T):
                q0 = t * 128
                m = min(128, W - q0)
                wci = q0 // 512  # chunk holding this tile's weight columns
                wo = q0 - 512 * wci
                ps = ppool.tile([128, W], f32, tag="ps")
                for ci, (c0, cn) in enumerate(CC):
                    has_diag = ci == wci
                    for k in range(KC):
                        nc.tensor.matmul(
                            ps[:m, c0:c0 + cn],
                            xt_sb[k][wci][:, wo:wo + m],
                            xt_sb[k][ci][:, :],
                            start=(k == 0),
                            stop=(k == KC - 1) and not has_diag)
                    if has_diag:
                        nc.tensor.matmul(
                            ps[:m, q0:q0 + m],
                            dei_sb[:, :m],
                            dei_sb[:, 128:128 + m],
                            start=False, stop=True)
                w1 = wpool.tile([128, W], f32, tag="w1")
                w2 = wpool.tile([128, W], f32, tag="w2")
                key = kpool.tile([128, W], f32, tag="key")
                if t == 0:
                    # halve the first tile's serial chain; put one key-add
                    # half on the (still idle) DVE
                    h = W // 2
                    for lo, hi in ((0, h), (h, W)):
                        # fp32 rounding at ulp(2^34)=2048: quantize to 0.5
                        nc.scalar.activation(
                            w1[:m, lo:hi], ps[:m, lo:hi],
                            mybir.ActivationFunctionType.Identity,
                            bias=biasp_sb[:m, t:t + 1], scale=8192.0)
                        nc.scalar.activation(
                            w2[:m, lo:hi], w1[:m, lo:hi],
                            mybir.ActivationFunctionType.Identity,
                            bias=bn34_sb[:m, :])
                    nc.gpsimd.tensor_add(
                        key[:m, 0:h], w2[:m, 0:h], iota_sb[:m, 0:h])
                    nc.vector.tensor_add(
                        key[:m, h:W], w2[:m, h:W], iota_sb[:m, h:W])
                else:
                    # fp32 rounding at ulp(2^34)=2048 quantizes to 0.5 steps
                    nc.scalar.activation(
                        w1[:m, :], ps[:m, :],
                        mybir.ActivationFunctionType.Identity,
                        bias=biasp_sb[:m, t:t + 1], scale=8192.0,
                    )
                    # w2 = w1 - 2^34 = q'*2048, exact
                    nc.scalar.activation(
                        w2[:m, :], w1[:m, :],
                        mybir.ActivationFunctionType.Identity,
                        bias=bn34_sb[:m, :])
                    # key = q'*2048 + (j - round(2*sq_j)*2048) : exact ints
                    nc.gpsimd.tensor_add(
                        key[:m, :], w2[:m, :], iota_sb[:m, :])
                kv = opool.tile([128, K], f32, tag="kv")
                for r in range(K // 8):
                    sl = slice(8 * r, 8 * r + 8)
                    nc.vector.max(out=kv[:m, sl], in_=key[:m, :])
                    if r < K // 8 - 1:
                        nc.vector.match_replace(
                            out=key[:m, :], in_to_replace=kv[:m, sl],
                            in_values=key[:m, :], imm_value=-BIG)
                nc.sync.dma_start(ok_d[q0:q0 + m, :], kv[:m, :])
    nc.finalize()
    return nc


def kernel(x, batch):
    global LAST_EXEC_NS
    import ml_dtypes
    from concourse.bass_utils import run_bass_kernel_spmd

    bf = ml_dtypes.bfloat16
    x = np.ascontiguousarray(np.asarray(x), dtype=np.float32)
    b = np.asarray(batch)
    N, D = x.shape
    bounds = np.searchsorted(b, np.arange(N_CORES + 1))
    sizes = np.diff(bounds)
    W = max(128, int(-(-sizes.max() // 8)) * 8)
    T = max(1, int(-(-sizes.max() // 128)))

    ckey = (W, T, D)
    if ckey not in _NC_CACHE:
        _NC_CACHE[ckey] = _build_nc(W, T, D)
    nc = _NC_CACHE[ckey]

    dei = np.zeros((128, 256), np.float32)
    dei[:, :128] = np.eye(128) * (-BIG / 2)
    dei[:, 128:] = np.eye(128)
    dei = dei.astype(bf)

    in_maps = []
    for c in range(N_CORES):
        s, e = int(bounds[c]), int(bounds[c + 1])
        n = e - s
        xc = x[s:e]
        xt = np.zeros((D, W), np.float32)
        xt[:, :n] = xc.T
        xt = xt.astype(bf)
        sq = np.einsum("ij,ij->i", xc, xc, dtype=np.float32)
        sq_pad = np.zeros(T * 128, np.float32)
        sq_pad[:n] = sq
        biasp = np.ascontiguousarray(
            (C34 + C23 - 4096.0 * sq_pad).astype(np.float32)
            .reshape(T, 128).T)
        # iota2[j] = j - round(2*sq_j)*2048 ; pad cols get -2^34-ish sink
        m2 = np.rint(2.0 * sq).astype(np.int64)
        io = np.full(W, -1e30, np.float64)
        io[:n] = np.arange(n) - m2 * 2048.0
        iota2 = io.astype(np.float32).reshape(1, W)
        in_maps.append({"xt": xt, "biasp": biasp, "dei": dei,
                        "iota2": iota2})

    trace = os.environ.get("KNN_TRACE", "0") == "1"
    res = run_bass_kernel_spmd(
        nc, in_maps, core_ids=list(range(N_CORES)), trace=trace)
    LAST_EXEC_NS = res.exec_time_ns

    out_d = np.empty((N, K), np.float32)
    out_i = np.empty((N, K), np.int32)
    for c in range(N_CORES):
        s, e = int(bounds[c]), int(bounds[c + 1])
        n = e - s
        if n == 0:
            continue
        kvi = res.results[c]["ok"][:n].astype(np.int64)  # exact integers
        j = kvi & 0x7FF
        q = (kvi >> 11) & 0xFFF
        out_d[s:e] = (2048.0 - 0.5 * q).astype(np.float32)
        out_i[s:e] = j + s
    return out_d, out_i
